# revision 19
# baseline (speedup 1.0000x reference)
"""CRF sequence-score kernel for Trainium2 (8 NeuronCores, SPMD).

Strategy (S-shard: core k owns s in [64k, 64k+64), all 512 batches):
  rows r = q*256 + x laid out as [q=128 partitions, x=256 cols];
  (s_local, b) = (q//2, 256*(q&1) + x).
  Per block x, ONE PSUM tile accumulates  M[m, t] = em[row_m, t]
  + T[t, tagnext_m]*masktr_m  via two PE matmuls:
    (1) lhsT = host-built fp8 onehot(tagnext)*masktr,  rhs = T^T fp8
    (2) lhsT = identity bf16,                          rhs = em chunk bf16
  PSUM -> SBUF bf16 in 4-block groups (Act), then ONE DVE
  scalar_tensor_tensor per block selects t = tag_m:
    macc[m, x] = em[row, tag] + T[tag, tagnext]*masktr.
  contrib = macc * maskem  (exact for step masks: maskem=0 => masktr=0);
  per-b reduction over s via parity matmul -> [2, 256].
  start/end terms: TWO matmuls  startv/endv (fp8 col) x host one-hot of
  tag[0,b] / tags[seq_end,b]  -> [1, 512]  (start real on core 0 only,
  end on core 7 only).
Host sums per-core outputs; score[b] = main[b//256, b%256] + se[0, b].
"""
import numpy as np

SEQ, BATCH, NTAGS = 512, 512, 128
NCORES = 8
SLICE = SEQ // NCORES            # 64 s-rows per core
NROWS = SLICE * BATCH            # 32768 rows per core
NBLK = NROWS // 128              # 256 blocks of 128 rows
P = 128
XC = 16                          # blocks per emissions chunk

_RUNNER = None


# ---------------------------------------------------------------------------
# walrus workaround: this build allows only ONE sync-wait per instruction.
def _install_tile_patch():
    import bass_rust
    import concourse.mybir as mybir
    import concourse.tile as tile
    from concourse.vector_clock import ScopedClock

    if getattr(tile.TileContext, "_crf_patched", False):
        return

    def _drain_and_barrier(self, tick_clock, wait_clock):
        nc = self.nc
        drain_inst = nc.sync.drain()
        wait_clock.add_sem_waits(
            drain_inst.ins, ScopedClock({None: tick_clock.global_clock})
        )
        si = drain_inst.ins.sync_info
        waits = list(si.on_wait) if si is not None and si.on_wait else []
        if len(waits) > 1:
            si.on_wait = waits[:1]
            for w in waits[1:]:
                extra = nc.sync.drain()
                if extra.ins.sync_info is None:
                    extra.ins.sync_info = bass_rust.SyncInfo(on_wait=[], on_update=[])
                extra.ins.sync_info.on_wait = [w]
        nc.all_engine_barrier()
        assert self.sems is not None
        popped = nc._tile_sem_poison_stack.pop()
        assert popped is self._sem_poison
        nc.clear_and_free_semaphores(list(self.sems.allocated().values()))
        nc.all_engine_barrier()

    orig_commit = tile.TileContext._commit_instruction

    def _commit(self, inst, lazy_reg_writes=True):
        si = getattr(inst, "sync_info", None)
        if (
            si is not None
            and si.on_wait
            and len(si.on_wait) > 1
            and inst.engine != mybir.EngineType.Unassigned
        ):
            waits = list(si.on_wait)
            si.on_wait = waits[:1]
            for w in waits[1:]:
                nop = mybir.InstNoOp(name=f"I-{self.nc.next_id()}", ins=[], outs=[])
                nop.engine = inst.engine
                nop.sync_info = bass_rust.SyncInfo(on_wait=[w], on_update=[])
                self._add_instruction(nop)
        return orig_commit(self, inst, lazy_reg_writes)

    tile.TileContext._drain_and_barrier = _drain_and_barrier
    tile.TileContext._commit_instruction = _commit
    tile.TileContext._crf_patched = True


# ---------------------------------------------------------------------------
def _build_nc():
    import concourse.bass as bass
    import concourse.mybir as mybir
    import concourse.tile as tile

    F32, I32, BF16 = mybir.dt.float32, mybir.dt.int32, mybir.dt.bfloat16
    FP8 = mybir.dt.float8e4
    AL = mybir.AluOpType

    nc = bass.Bass()
    em = nc.declare_dram_parameter("em", [NROWS * NTAGS], BF16, isOutput=False)
    oht_d = nc.declare_dram_parameter("oht", [P * NROWS], FP8, isOutput=False)
    ttab_d = nc.declare_dram_parameter("ttab", [P * NTAGS], FP8, isOutput=False)
    tagt_d = nc.declare_dram_parameter("tagt", [P * NBLK], BF16, isOutput=False)
    mem_d = nc.declare_dram_parameter("memf", [P * NBLK], F32, isOutput=False)
    par_d = nc.declare_dram_parameter("par", [P * 2], F32, isOutput=False)
    seo_d = nc.declare_dram_parameter("seoht", [P * 1024], FP8, isOutput=False)
    sev_d = nc.declare_dram_parameter("sevals", [P * 2], FP8, isOutput=False)
    out_m = nc.declare_dram_parameter("out_m", [2, NBLK], F32, isOutput=True)
    out_se = nc.declare_dram_parameter("out_se", [1, BATCH], F32, isOutput=True)

    with tile.TileContext(nc) as tc:
        with tc.tile_pool(name="sbuf", bufs=1) as sb, \
             tc.tile_pool(name="psum", bufs=1, space="PSUM") as ps, \
             tc.tile_pool(name="emp", bufs=3) as emp:
            # ---- constants / staging
            iota_i = sb.tile([P, NTAGS], I32, name="iota_i")
            nc.gpsimd.iota(iota_i[:], pattern=[[1, NTAGS]], base=0, channel_multiplier=0)
            iota = sb.tile([P, NTAGS], BF16, name="iota")
            nc.vector.tensor_copy(out=iota[:], in_=iota_i[:])
            iop_i = sb.tile([P, 1], I32, name="iop_i")
            nc.gpsimd.iota(iop_i[:], pattern=[[0, 1]], base=0, channel_multiplier=1)
            iop = sb.tile([P, 1], F32, name="iop")
            nc.vector.tensor_copy(out=iop[:], in_=iop_i[:])
            ident = sb.tile([P, NTAGS], BF16, name="ident")
            nc.vector.tensor_scalar(out=ident[:], in0=iota[:], scalar1=iop[:],
                                    scalar2=None, op0=AL.is_equal)

            ttab = sb.tile([P, NTAGS], FP8, name="ttab")
            nc.sync.dma_start(out=ttab[:], in_=ttab_d[:].rearrange("(p t) -> p t", p=P))
            ohts = sb.tile([P, NROWS], FP8, name="ohts")
            tagt = sb.tile([P, NBLK], BF16, name="tagt")
            nc.sync.dma_start(out=tagt[:], in_=tagt_d[:].rearrange("(p x) -> p x", p=P))
            memf = sb.tile([P, NBLK], F32, name="memf")
            par = sb.tile([P, 2], F32, name="par")
            seoht = sb.tile([P, 1024], FP8, name="seoht")
            sevals = sb.tile([P, 2], FP8, name="sevals")

            # ---- main loop (stt's run one group behind the Act copy)
            macc = sb.tile([P, NBLK], F32, name="macc")
            junks = [sb.tile([P, NTAGS], BF16, name=f"junk{i}", tag=f"jk{i}")
                     for i in range(8)]
            tnsbs = [sb.tile([P, 4 * NTAGS], BF16, name=f"tnsb{i}", tag=f"tn{i}")
                     for i in range(6)]
            emch = None
            tn_ps = None

            def stt_group(g):
                tnsb = tnsbs[g % 6]
                for xx in range(g * 4, g * 4 + 4):
                    o2 = (xx % 4) * NTAGS
                    nc.vector.scalar_tensor_tensor(
                        out=junks[xx % 8][:], in0=iota[:],
                        scalar=tagt[:, xx:xx + 1],
                        in1=tnsb[:, o2:o2 + NTAGS],
                        op0=AL.is_equal, op1=AL.mult,
                        accum_out=macc[:, xx:xx + 1],
                    )

            for x in range(NBLK):
                d, sub = x // XC, x % XC
                if sub == 0:
                    nc.sync.dma_start(
                        out=ohts[:, d * XC * P:(d + 1) * XC * P],
                        in_=oht_d[:].rearrange("(q r) -> q r", q=P)
                            [:, d * XC * P:(d + 1) * XC * P],
                    )
                    emch = emp.tile([P, XC * NTAGS], BF16, name=f"emch{d}", tag="emch")
                    nc.sync.dma_start(
                        out=emch[:],
                        in_=em[:].rearrange("(q x t) -> q (x t)", q=P, x=NBLK)
                            [:, d * XC * NTAGS:(d + 1) * XC * NTAGS],
                    )
                    if d == 0:
                        nc.sync.dma_start(out=memf[:],
                                          in_=mem_d[:].rearrange("(p x) -> p x", p=P))
                        nc.sync.dma_start(out=par[:],
                                          in_=par_d[:].rearrange("(p h) -> p h", p=P))
                        nc.sync.dma_start(out=seoht[:],
                                          in_=seo_d[:].rearrange("(p b) -> p b", p=P))
                        nc.sync.dma_start(out=sevals[:],
                                          in_=sev_d[:].rearrange("(p c) -> p c", p=P))
                g, off = x // 4, (x % 4) * NTAGS
                if x % 4 == 0:
                    tn_ps = ps.tile([P, 4 * NTAGS], F32, name=f"tnps{g % 4}",
                                    tag=f"tb{g % 4}")
                nc.tensor.matmul(
                    out=tn_ps[:, off:off + NTAGS],
                    lhsT=ohts[:, x * P:(x + 1) * P],
                    rhs=ttab[:], start=True, stop=False,
                )
                nc.tensor.matmul(
                    out=tn_ps[:, off:off + NTAGS],
                    lhsT=ident[:],
                    rhs=emch[:, sub * NTAGS:(sub + 1) * NTAGS],
                    start=False, stop=True,
                )
                if x % 4 == 3:
                    nc.scalar.copy(out=tnsbs[g % 6][:], in_=tn_ps[:])
                    if g > 0:
                        stt_group(g - 1)
                if x == 41:
                    # start/end terms mid-loop: [1,512] = sv x oht0 + ev x ohtE
                    se_ps = ps.tile([1, BATCH], F32, name="se_ps", tag="pse")
                    nc.tensor.matmul(out=se_ps[:], lhsT=sevals[:, 0:1],
                                     rhs=seoht[:, 0:BATCH], start=True, stop=False)
                    nc.tensor.matmul(out=se_ps[:], lhsT=sevals[:, 1:2],
                                     rhs=seoht[:, BATCH:2 * BATCH],
                                     start=False, stop=True)
                    se_sb = sb.tile([1, BATCH], F32, name="se_sb")
                    nc.scalar.copy(out=se_sb[:], in_=se_ps[:])
                    nc.sync.dma_start(out=out_se[:], in_=se_sb[:])
            stt_group(NBLK // 4 - 1)

            # ---- epilogue: contrib = macc*memf; parity-sum over q
            contrib = sb.tile([P, NBLK], F32, name="contrib")
            nc.vector.tensor_tensor(out=contrib[:], in0=macc[:], in1=memf[:],
                                    op=AL.mult)
            mainp = ps.tile([2, NBLK], F32, name="mainp", tag="pse")
            nc.tensor.matmul(out=mainp[:], lhsT=par[:], rhs=contrib[:],
                             start=True, stop=True)
            mains = sb.tile([2, NBLK], F32, name="mains")
            nc.vector.tensor_copy(out=mains[:], in_=mainp[:])
            nc.sync.dma_start(out=out_m[:], in_=mains[:])

    return nc


# ---------------------------------------------------------------------------
def _make_runner(nc, n_cores=8):
    import jax
    from jax.sharding import Mesh, PartitionSpec
    from jax.experimental.shard_map import shard_map
    import concourse.mybir as mybir
    from concourse import bass2jax

    bass2jax.install_neuronx_cc_hook()
    partition_name = nc.partition_id_tensor.name if nc.partition_id_tensor else None
    in_names, out_names, out_avals, zero_outs = [], [], [], []
    for alloc in nc.m.functions[0].allocations:
        if not isinstance(alloc, mybir.MemoryLocationSet):
            continue
        name = alloc.memorylocations[0].name
        if alloc.kind == "ExternalInput":
            if name != partition_name:
                in_names.append(name)
        elif alloc.kind == "ExternalOutput":
            shape = tuple(alloc.tensor_shape)
            dtype = mybir.dt.np(alloc.dtype)
            out_names.append(name)
            out_avals.append(jax.core.ShapedArray(shape, dtype))
            zero_outs.append(np.zeros(shape, dtype))
    n_params = len(in_names)
    all_in_names = list(in_names) + list(out_names)
    if partition_name is not None:
        all_in_names.append(partition_name)

    def _body(*args):
        operands = list(args)
        if partition_name is not None:
            operands.append(bass2jax.partition_id_tensor())
        outs = bass2jax._bass_exec_p.bind(
            *operands, out_avals=tuple(out_avals), in_names=tuple(all_in_names),
            out_names=tuple(out_names), lowering_input_output_aliases=(),
            sim_require_finite=True, sim_require_nnan=True, nc=nc,
        )
        return tuple(outs)

    devices = jax.devices()[:n_cores]
    mesh = Mesh(np.asarray(devices), ("core",))
    n_outs = len(out_names)
    jitted = jax.jit(
        shard_map(_body, mesh=mesh,
                  in_specs=(PartitionSpec("core"),) * (n_params + n_outs),
                  out_specs=(PartitionSpec("core"),) * n_outs, check_rep=False),
        keep_unused=True,
    )

    def run(in_maps):
        per_core = [[np.asarray(m[nm]) for nm in in_names] for m in in_maps]
        concat_in = [np.concatenate([per_core[c][i] for c in range(n_cores)], axis=0)
                     for i in range(n_params)]
        concat_zero = [np.concatenate([z] * n_cores, axis=0) for z in zero_outs]
        outs = [np.asarray(o) for o in jitted(*concat_in, *concat_zero)]
        results = []
        for c in range(n_cores):
            d = {}
            for i, nm in enumerate(out_names):
                per = outs[i].shape[0] // n_cores
                d[nm] = outs[i][c * per:(c + 1) * per]
            results.append(d)
        return results

    return run


def _get_runner():
    global _RUNNER
    if _RUNNER is None:
        _install_tile_patch()
        _RUNNER = _make_runner(_build_nc(), NCORES)
    return _RUNNER


# ---------------------------------------------------------------------------
def make_in_maps(emissions, tags, mask, start_transitions, end_transitions,
                 transitions):
    import ml_dtypes
    BF16, FP8 = ml_dtypes.bfloat16, ml_dtypes.float8_e4m3

    emissions = np.asarray(emissions, dtype=np.float32)
    tg = np.asarray(tags).astype(np.int64)
    msk = np.asarray(mask).astype(np.int64)

    ttab = np.ascontiguousarray(np.asarray(transitions, np.float32).T).astype(FP8)
    par = np.zeros((P, 2), np.float32)
    par[0::2, 0] = 1.0
    par[1::2, 1] = 1.0

    # start/end one-hot [128, 1024]: cols 0:512 onehot(tag[0,b]),
    # cols 512:1024 onehot(tags[seq_end_b, b])
    bidx = np.arange(BATCH)
    seq_end = msk.sum(axis=0).astype(np.int64) - 1
    last_tag = tg[seq_end, bidx]
    oht0 = np.zeros((P, BATCH), np.float32)
    oht0[tg[0], bidx] = 1.0
    ohtE = np.zeros((P, BATCH), np.float32)
    ohtE[last_tag, bidx] = 1.0
    zero_se = np.zeros((P, BATCH), np.float32)
    sevals = np.stack([np.asarray(start_transitions, np.float32),
                       np.asarray(end_transitions, np.float32)], axis=1)  # [128,2]
    sev8 = sevals.astype(FP8)

    # (q, x) grids: s_local = q//2, b = 256*(q&1) + x
    q = np.arange(P)[:, None]
    x = np.arange(NBLK)[None, :]
    slg = q // 2
    bg = 256 * (q & 1) + x

    in_maps = []
    for k in range(NCORES):
        s0 = k * SLICE
        sgb = np.broadcast_to(s0 + slg, (P, NBLK))
        tag = tg[sgb, bg]
        em_k = emissions[s0:s0 + SLICE].astype(BF16).reshape(-1)

        maskem = (msk[sgb, bg] != 0).astype(np.float32)
        if k == 0:
            maskem[sgb == 0] = 1.0  # reference counts emit[0] unconditionally

        # trans one-hots: oht[p, x*128 + q] = (tagnext[q,x]==p) * masktr[q,x]
        last = sgb == SEQ - 1
        snext = np.minimum(sgb + 1, SEQ - 1)
        masktr = np.where(last, 0, msk[snext, bg]).astype(np.float32)
        tagn = np.where(last, 0, tg[snext, bg])
        col = x * P + q
        oht = np.zeros((P, NROWS), dtype=np.float32)
        oht[tagn.reshape(-1), col.reshape(-1)] = masktr.reshape(-1)

        seoht = np.concatenate(
            [oht0 if k == 0 else zero_se,
             ohtE if k == NCORES - 1 else zero_se], axis=1)  # [128, 1024]

        in_maps.append({
            "em": em_k,
            "oht": oht.astype(FP8).reshape(-1),
            "ttab": ttab.reshape(-1),
            "tagt": tag.astype(BF16).reshape(-1),
            "memf": maskem.reshape(-1),
            "par": par.reshape(-1),
            "seoht": seoht.astype(FP8).reshape(-1),
            "sevals": sev8.reshape(-1),
        })
    return in_maps


def kernel(emissions, tags, mask, start_transitions, end_transitions,
           transitions):
    run = _get_runner()
    in_maps = make_in_maps(emissions, tags, mask, start_transitions,
                           end_transitions, transitions)
    results = run(in_maps)
    main = np.zeros((2, NBLK), np.float64)
    se = np.zeros((1, BATCH), np.float64)
    for r_ in results:
        main += r_["out_m"].astype(np.float64)
        se += r_["out_se"].astype(np.float64)
    score = main.reshape(BATCH) + se[0]              # b = h*256 + x
    return score.astype(np.float32)
